# revision 1
# baseline (speedup 1.0000x reference)
"""Trainium2 Bass kernel for ExampleGuidedAttention (N=8, C=256, H=W=64).

Data-parallel over batch N across 8 NeuronCores; each core computes one
batch element's full guided attention.

Algorithm notes (per core):
  q = conv_w @ src_pix                      [64, 4096]   (PE, bf16)
  S^T[j,i] = sum_o q[o,j] q[o,i]            (PE, bf16; S symmetric; two
             j-blocks packed in the 128x128 array via tile_position
             (row groups 0-63 / 64-127) since the contraction is only 64)
  F[j,i] = exp(S^T[j,i] - 64)               (ACT; global shift keeps fp32
             exp in range -- softmax ratio unchanged; diag scores are
             chi2(64) so they reach ~120)
  Z[p]   = sum_i F (free-dim reduce of the symmetric tiles on DVE)
  O[c,i] = sum_j pixT[j,c] * F[j,i]         (PE, bf16, natural layout)
  out    = [ (1-m)*ref_att*invZ + m*ref ; src_att*invZ ]

Finalize for slices 0..6 runs on DVE while the PE is still doing slice
7's apply matmuls; slice 7's copy-out doubles as its normalize.
"""

import numpy as np

import concourse.bass as bass
import concourse.mybir as mybir
import concourse.tile as tile
from concourse import bacc, bass_utils
from concourse.bass import ts
from concourse.masks import make_identity

P = 128
C = 256          # feature channels
CQ = 64          # query channels
HW = 4096        # pixels per image
NB = HW // P     # 32 pixel blocks (contraction chunks)
SLICE = 512
NS = HW // SLICE  # 8 output column slices
NCORES = 8

F32 = mybir.dt.float32
BF16 = mybir.dt.bfloat16
EXP = mybir.ActivationFunctionType.Exp
AX_X = mybir.AxisListType.X


def _build_body(tc, src, ref, mask, wT, out):
    nc = tc.nc
    src_r = src.ap().rearrange("(ci p) j -> p ci j", p=P)   # [128, 2, 4096]
    ref_r = ref.ap().rearrange("(ci p) j -> p ci j", p=P)
    wT_r = wT.ap().rearrange("(ci p) o -> p ci o", p=P)     # [128, 2, 64]
    out_r = out.ap().rearrange("(cb p) j -> cb p j", p=P)   # [4, 128, 4096]

    with (
        tc.tile_pool(name="persist", bufs=1) as persist,
        tc.tile_pool(name="ps_s", bufs=4, space="PSUM") as ps_s,
        tc.tile_pool(name="ps_o", bufs=4, space="PSUM") as ps_o,
        tc.tile_pool(name="dram", bufs=1, space="DRAM") as dram,
    ):
        # bf16 ref copy doubles as the blend operand (saves the fp32 copy)
        refb = persist.tile([P, 2, HW], BF16)
        # q duplicated into both partition halves so scores matmuls can be
        # row-packed: tile at rows 0-63 and rows 64-127 run concurrently.
        q2 = persist.tile([P, HW], BF16)
        pixT_src = persist.tile([P, NB, C], BF16)
        pixT_ref = persist.tile([P, NB, C], BF16)
        wT_sb = persist.tile([P, 2, CQ], BF16)
        zpart = persist.tile([P, NB, NS], F32)
        z_all = persist.tile([P, NB], F32)
        invz = persist.tile([P, NB], F32)
        mask_rep = persist.tile([P, HW], F32)
        invz_rep = persist.tile([P, HW], F32)
        exp_bias = persist.tile([P, 1], F32)
        ident = persist.tile([P, P], F32)
        invz_T = persist.tile([NB, P], F32)
        zrow = dram.tile([HW], F32)
        nc.vector.memset(exp_bias, -64.0)
        make_identity(nc, ident)

        nc.sync.dma_start(out=wT_sb, in_=wT_r)
        for s in range(NS):
            nc.scalar.dma_start(
                out=mask_rep[:, ts(s, SLICE)],
                in_=mask.ap()[ts(s, SLICE)].partition_broadcast(P),
            )

        with tc.tile_pool(name="early", bufs=1) as early:
            # PE warmup: back-to-back matmuls on zeroed data latch the HAM
            # clock gate to 8/8 (2.4 GHz) while input DMAs stream in.
            warm_sb = early.tile([P, SLICE], BF16)
            nc.vector.memset(warm_sb, 0.0)
            warm_ps = ps_s.tile([P, SLICE], F32, name="warm_ps", tag="pss")
            for _ in range(18):
                nc.tensor.matmul(
                    warm_ps, warm_sb[:, 0:P], warm_sb, start=True, stop=True
                )
            srcb = early.tile([P, 2, HW], BF16)
            # src casts first (conv + src transpose depend on them)
            for ci in range(2):
                for s in range(NS):
                    sl = ts(s, SLICE)
                    nc.gpsimd.dma_start(out=srcb[:, ci, sl], in_=src_r[:, ci, sl])
            for ci in range(2):
                for s in range(NS):
                    sl = ts(s, SLICE)
                    nc.gpsimd.dma_start(out=refb[:, ci, sl], in_=ref_r[:, ci, sl])
            # XBAR transposes on two HWDGE queues, split in j-halves so each
            # can start as soon as half the casts have landed:
            # pixT[p, b, c] = pix[c, b*128+p]
            for ci in range(2):
                cs = slice(ci * P, (ci + 1) * P)
                for h in range(2):
                    jh = slice(h * (HW // 2), (h + 1) * (HW // 2))
                    bh = slice(h * (NB // 2), (h + 1) * (NB // 2))
                    nc.sync.dma_start_transpose(
                        out=pixT_src[:, bh, cs], in_=srcb[:, ci, jh]
                    )
                    nc.scalar.dma_start_transpose(
                        out=pixT_ref[:, bh, cs], in_=refb[:, ci, jh]
                    )
            # 1x1 conv: q = wT.T @ src_pix; write q into both partition halves
            for s in range(NS):
                sl = ts(s, SLICE)
                psq = ps_s.tile([CQ, SLICE], F32, name="psq", tag="pss")
                for ci in range(2):
                    nc.tensor.matmul(
                        psq,
                        wT_sb[:, ci, :],
                        srcb[:, ci, sl],
                        start=(ci == 0),
                        stop=(ci == 1),
                    )
                nc.vector.tensor_copy(out=q2[0:CQ, sl], in_=psq)
                nc.vector.tensor_copy(out=q2[CQ:P, sl], in_=psq)

        def scores_and_exp(s, f_sb):
            sl = ts(s, SLICE)
            for jp in range(NB // 2):
                jb0, jb1 = 2 * jp, 2 * jp + 1
                pss0 = ps_s.tile([P, SLICE], F32, name="pss0", tag="pss")
                pss1 = ps_s.tile([P, SLICE], F32, name="pss1", tag="pss")
                nc.tensor.matmul(
                    pss0, q2[0:CQ, ts(jb0, P)], q2[0:CQ, sl],
                    start=True, stop=True, tile_position=(0, 0),
                )
                nc.tensor.matmul(
                    pss1, q2[CQ:P, ts(jb1, P)], q2[CQ:P, sl],
                    start=True, stop=True, tile_position=(CQ, 0),
                )
                for jb, pss in ((jb0, pss0), (jb1, pss1)):
                    nc.scalar.activation(
                        out=f_sb[:, jb, :], in_=pss, func=EXP, bias=exp_bias
                    )
            # reduce in groups of 8 j-blocks: big enough to amortize DVE op
            # overhead, small enough not to serialize the DVE queue
            for g in range(NB // 8):
                nc.vector.reduce_sum(
                    out=zpart[:, ts(g, 8), s : s + 1],
                    in_=f_sb[:, ts(g, 8), :],
                    axis=AX_X,
                )

        def apply_mm(s, f_sb):
            psos = [
                ps_o.tile([P, SLICE], F32, name=f"pso{cb}", tag="pso")
                for cb in range(4)
            ]
            for jb in range(NB):
                for cb in range(4):
                    pt = pixT_src if cb < 2 else pixT_ref
                    lhs = pt[:, jb, (cb % 2) * P : (cb % 2 + 1) * P]
                    nc.tensor.matmul(
                        psos[cb], lhs, f_sb[:, jb, :],
                        start=(jb == 0), stop=(jb == NB - 1),
                    )
            return psos

        def copy_out(s, psos):
            sl = ts(s, SLICE)
            for cb in range(4):
                nc.vector.tensor_copy(out=o_sb[:, cb, sl], in_=psos[cb])

        def finalize(lo, hi, dma_engines, skip_norm=False):
            """Normalize + blend + store for pixel columns [lo:hi).

            All elementwise work stays on DVE: GpSimd shares (and locks) the
            DVE SBUF port, so splitting across both engines makes each ~3x
            slower with no net gain.
            """
            r = slice(lo, hi)
            if not skip_norm:
                for cb in range(4):
                    nc.vector.tensor_mul(
                        o_sb[:, cb, r], o_sb[:, cb, r], invz_rep[:, r]
                    )
            for cb in (2, 3):
                ci = cb - 2
                nc.vector.tensor_sub(tmp[:, r], refb[:, ci, r], o_sb[:, cb, r])
                nc.vector.tensor_mul(tmp[:, r], tmp[:, r], mask_rep[:, r])
                nc.vector.tensor_add(o_sb[:, cb, r], o_sb[:, cb, r], tmp[:, r])
            # out rows: [flow(=cb2,3), src_att(=cb0,1)]
            for k, cb in enumerate([2, 3, 0, 1]):
                eng = dma_engines[k % len(dma_engines)]
                for s in range(lo // SLICE, (hi + SLICE - 1) // SLICE):
                    sl = ts(s, SLICE)
                    eng.dma_start(out=out_r[k, :, sl], in_=o_sb[:, cb, sl])

        with tc.tile_pool(name="obuf", bufs=1) as obuf:
            o_sb = obuf.tile([P, 4, HW], F32)
            tmp = obuf.tile([P, HW], F32)
            with tc.tile_pool(name="fbuf", bufs=1) as fbuf:
                for s in range(NS - 1):
                    f_sb = fbuf.tile([P, NB, SLICE], BF16, name="f_sb", tag="f")
                    scores_and_exp(s, f_sb)
                    psos = apply_mm(s, f_sb)
                    copy_out(s, psos)
                # slice 7: scores/exp first so Z completes, then overlap
                # finalize(0..6) with slice 7's apply matmuls
                s7 = NS - 1
                f_sb7 = fbuf.tile([P, NB, SLICE], BF16, name="f_sb", tag="f")
                scores_and_exp(s7, f_sb7)
                nc.vector.reduce_sum(out=z_all, in_=zpart, axis=AX_X)
                nc.vector.reciprocal(out=invz, in_=z_all)
                # broadcast invz[pixel] across the 128 channel partitions:
                # PE-transpose [128p, 32b] -> [32b, 128p], bounce through
                # DRAM contiguously, then partition-broadcast back (a direct
                # scatter of the untransposed layout costs ~16us)
                ps_t = ps_s.tile([NB, P], F32, name="ps_t", tag="pss")
                nc.tensor.transpose(ps_t, invz[:, :], ident)
                nc.vector.tensor_copy(out=invz_T, in_=ps_t)
                nc.sync.dma_start(
                    out=zrow.rearrange("(b q) -> b q", q=P), in_=invz_T
                )
                for s in range(NS):
                    sl = ts(s, SLICE)
                    nc.sync.dma_start(
                        out=invz_rep[:, sl], in_=zrow[sl].partition_broadcast(P)
                    )
                psos7 = apply_mm(s7, f_sb7)
                finalize(0, (NS - 1) * SLICE, [nc.sync, nc.scalar])
                # slice-7 copy-out doubles as its normalize (Z is ready now)
                sl7 = ts(s7, SLICE)
                for cb in range(4):
                    nc.vector.tensor_mul(
                        o_sb[:, cb, sl7], psos7[cb], invz_rep[:, sl7]
                    )
                finalize((NS - 1) * SLICE, HW, [nc.scalar, nc.sync], skip_norm=True)


def build():
    nc = bacc.Bacc(
        "TRN2",
        target_bir_lowering=False,
        debug=False,
        enable_asserts=False,
        num_devices=NCORES,
    )
    src = nc.dram_tensor("src", (C, HW), F32, kind="ExternalInput")
    ref = nc.dram_tensor("ref", (C, HW), F32, kind="ExternalInput")
    mask = nc.dram_tensor("mask", (HW,), F32, kind="ExternalInput")
    wT = nc.dram_tensor("wT", (C, CQ), BF16, kind="ExternalInput")
    out = nc.dram_tensor("out", (2 * C, HW), F32, kind="ExternalOutput")
    with tile.TileContext(nc) as tc:
        _build_body(tc, src, ref, mask, wT, out)
    nc.compile()
    return nc


_CACHE = {}


def _get_nc():
    if "nc" not in _CACHE:
        _CACHE["nc"] = build()
    return _CACHE["nc"]


def _in_maps(src_mask, src_feature, ref_feature, conv_w):
    import ml_dtypes

    n_batch = src_feature.shape[0]
    wT = np.ascontiguousarray(
        np.asarray(conv_w, dtype=np.float32).T.astype(ml_dtypes.bfloat16)
    )
    maps = []
    for n in range(n_batch):
        maps.append(
            {
                "src": np.ascontiguousarray(
                    np.asarray(src_feature[n], dtype=np.float32).reshape(C, HW)
                ),
                "ref": np.ascontiguousarray(
                    np.asarray(ref_feature[n], dtype=np.float32).reshape(C, HW)
                ),
                "mask": np.ascontiguousarray(
                    np.asarray(src_mask[n], dtype=np.float32).reshape(HW)
                ),
                "wT": wT,
            }
        )
    return maps


def _install_ntff_hook():
    """The agent image's antenv lacks axon_hooks; recreate it so
    run_bass_kernel_spmd(trace=True) can capture NTFF profiles."""
    import sys
    import types

    if "antenv.axon_hooks" in sys.modules:
        return
    import antenv
    from trn_agent_boot.trn_boot import _ntff_profile_via_ctypes

    hook = _ntff_profile_via_ctypes("/opt/axon/libaxon_pjrt.so")
    mod = types.ModuleType("antenv.axon_hooks")
    mod._hook = hook
    mod.set_axon_ntff_profile_hook = lambda h: setattr(mod, "_hook", h)
    mod.get_axon_ntff_profile_hook = lambda: mod._hook
    sys.modules["antenv.axon_hooks"] = mod
    antenv.axon_hooks = mod


def run(src_mask, src_feature, ref_feature, conv_w, trace=False):
    """Run on 8 NeuronCores. Returns (output [N,2C,H,W], BassKernelResults)."""
    n_batch, c, h, w = src_feature.shape
    if trace:
        _install_ntff_hook()
    nc = _get_nc()
    maps = _in_maps(src_mask, src_feature, ref_feature, conv_w)
    res = bass_utils.run_bass_kernel_spmd(
        nc, maps, core_ids=list(range(NCORES)), trace=trace
    )
    out = np.stack([r["out"] for r in res.results], axis=0)
    return out.reshape(n_batch, 2 * c, h, w).astype(np.float32), res


def kernel(src_mask, src_feature, ref_feature, conv_w):
    out, _ = run(src_mask, src_feature, ref_feature, conv_w)
    return out



# revision 16
# speedup vs baseline: 1.0192x; 1.0192x over previous
"""Trainium2 Bass kernel for ExampleGuidedAttention (N=8, C=256, H=W=64).

Data-parallel over batch N across 8 NeuronCores; each core computes one
batch element's full guided attention.

Algorithm notes (per core):
  q = conv_w @ src_pix                      [64, 4096]   (PE, bf16)
  S^T[j,i] = sum_o q[o,j] q[o,i]            (PE, bf16; S symmetric; two
             j-blocks packed in the 128x128 array via tile_position
             (row groups 0-63 / 64-127) since the contraction is only 64)
  F[j,i] = exp(S^T[j,i] - 64)               (ACT; global shift keeps fp32
             exp in range -- softmax ratio unchanged; diag scores are
             chi2(64) so they reach ~120)
  Z[p]   = sum_i F (free-dim reduce of the symmetric tiles on DVE)
  O[c,i] = sum_j pixT[j,c] * F[j,i]         (PE, bf16, natural layout)
  out    = [ (1-m)*ref_att*invZ + m*ref ; src_att*invZ ]

Finalize for slices 0..6 runs on DVE while the PE is still doing slice
7's apply matmuls; slice 7's copy-out doubles as its normalize.
"""

import numpy as np

import concourse.bass as bass
import concourse.mybir as mybir
import concourse.tile as tile
from concourse import bacc, bass_utils
from concourse.bass import ts
from concourse.masks import make_identity

P = 128
C = 256          # feature channels
CQ = 64          # query channels
HW = 4096        # pixels per image
NB = HW // P     # 32 pixel blocks (contraction chunks)
SLICE = 512
NS = HW // SLICE  # 8 output column slices
NCORES = 8

F32 = mybir.dt.float32
BF16 = mybir.dt.bfloat16
EXP = mybir.ActivationFunctionType.Exp
AX_X = mybir.AxisListType.X


def _build_body(tc, src, ref, mask, wT, out):
    nc = tc.nc
    src_r = src.ap().rearrange("(ci p) j -> p ci j", p=P)   # [128, 2, 4096]
    ref_r = ref.ap().rearrange("(ci p) j -> p ci j", p=P)
    wT_r = wT.ap().rearrange("(ci p) o -> p ci o", p=P)     # [128, 2, 64]
    out_r = out.ap().rearrange("(cb p) j -> cb p j", p=P)   # [4, 128, 4096]

    with (
        tc.tile_pool(name="persist", bufs=1) as persist,
        tc.tile_pool(name="ps_s", bufs=4, space="PSUM") as ps_s,
        tc.tile_pool(name="ps_o", bufs=4, space="PSUM") as ps_o,
        tc.tile_pool(name="dram", bufs=1, space="DRAM") as dram,
    ):
        # bf16 ref copy doubles as the blend operand (saves the fp32 copy)
        refb = persist.tile([P, 2, HW], BF16)
        # q duplicated into both partition halves so scores matmuls can be
        # row-packed: tile at rows 0-63 and rows 64-127 run concurrently.
        q2 = persist.tile([P, HW], BF16)
        pixT_src = persist.tile([P, NB, C], BF16)
        pixT_ref = persist.tile([P, NB, C], BF16)
        wT_sb = persist.tile([P, 2, CQ], BF16)
        zpart = persist.tile([P, NB, NS], F32)
        z_all = persist.tile([P, NB], F32)
        invz = persist.tile([P, NB], F32)
        mask_rep = persist.tile([P, HW], F32)
        invz_rep = persist.tile([P, HW], F32)
        exp_bias = persist.tile([P, 1], F32)
        ident = persist.tile([P, P], F32)
        invz_T = persist.tile([NB, P], F32)
        zrow = dram.tile([HW], F32)
        nc.vector.memset(exp_bias, -64.0)
        make_identity(nc, ident)

        nc.sync.dma_start(out=wT_sb, in_=wT_r)
        for s in range(NS):
            nc.scalar.dma_start(
                out=mask_rep[:, ts(s, SLICE)],
                in_=mask.ap()[ts(s, SLICE)].partition_broadcast(P),
            )

        with tc.tile_pool(name="early", bufs=1) as early:
            # PE warmup: back-to-back matmuls on zeroed data latch the HAM
            # clock gate to 8/8 (2.4 GHz) while input DMAs stream in.
            warm_sb = early.tile([P, SLICE], BF16)
            nc.vector.memset(warm_sb, 0.0)
            warm_ps = ps_s.tile([P, SLICE], F32, name="warm_ps", tag="pss")
            for _ in range(18):
                nc.tensor.matmul(
                    warm_ps, warm_sb[:, 0:P], warm_sb, start=True, stop=True
                )
            srcb = early.tile([P, 2, HW], BF16)
            # src casts first (conv + src transpose depend on them)
            for ci in range(2):
                for h in range(4):
                    jh = slice(h * (HW // 4), (h + 1) * (HW // 4))
                    nc.gpsimd.dma_start(out=srcb[:, ci, jh], in_=src_r[:, ci, jh])
            for ci in range(2):
                for h in range(4):
                    jh = slice(h * (HW // 4), (h + 1) * (HW // 4))
                    nc.gpsimd.dma_start(out=refb[:, ci, jh], in_=ref_r[:, ci, jh])
            # XBAR transposes on two HWDGE queues, split in j-halves so each
            # can start as soon as half the casts have landed:
            # pixT[p, b, c] = pix[c, b*128+p]
            for ci in range(2):
                cs = slice(ci * P, (ci + 1) * P)
                for h in range(2):
                    jh = slice(h * (HW // 2), (h + 1) * (HW // 2))
                    bh = slice(h * (NB // 2), (h + 1) * (NB // 2))
                    nc.sync.dma_start_transpose(
                        out=pixT_src[:, bh, cs], in_=srcb[:, ci, jh]
                    )
                    nc.scalar.dma_start_transpose(
                        out=pixT_ref[:, bh, cs], in_=refb[:, ci, jh]
                    )
            # 1x1 conv: q = wT.T @ src_pix; write q into both partition halves
            for s in range(NS):
                sl = ts(s, SLICE)
                psq = ps_s.tile([CQ, SLICE], F32, name="psq", tag="pss")
                for ci in range(2):
                    nc.tensor.matmul(
                        psq,
                        wT_sb[:, ci, :],
                        srcb[:, ci, sl],
                        start=(ci == 0),
                        stop=(ci == 1),
                    )
                nc.vector.tensor_copy(out=q2[0:CQ, sl], in_=psq)
                nc.vector.tensor_copy(out=q2[CQ:P, sl], in_=psq)

        def scores_and_exp(s, f_sb):
            sl = ts(s, SLICE)
            for jp in range(NB // 2):
                jb0, jb1 = 2 * jp, 2 * jp + 1
                pss0 = ps_s.tile([P, SLICE], F32, name="pss0", tag="pss")
                pss1 = ps_s.tile([P, SLICE], F32, name="pss1", tag="pss")
                nc.tensor.matmul(
                    pss0, q2[0:CQ, ts(jb0, P)], q2[0:CQ, sl],
                    start=True, stop=True, tile_position=(0, 0),
                )
                nc.tensor.matmul(
                    pss1, q2[CQ:P, ts(jb1, P)], q2[CQ:P, sl],
                    start=True, stop=True, tile_position=(CQ, 0),
                )
                for jb, pss in ((jb0, pss0), (jb1, pss1)):
                    nc.scalar.activation(
                        out=f_sb[:, jb, :], in_=pss, func=EXP, bias=exp_bias
                    )
            # reduce in groups of 8 j-blocks: big enough to amortize DVE op
            # overhead, small enough not to serialize the DVE queue
            for g in range(NB // 8):
                nc.vector.reduce_sum(
                    out=zpart[:, ts(g, 8), s : s + 1],
                    in_=f_sb[:, ts(g, 8), :],
                    axis=AX_X,
                )

        def apply_mm(s, f_sb):
            psos = [
                ps_o.tile([P, SLICE], F32, name=f"pso{cb}", tag="pso")
                for cb in range(4)
            ]
            for jb in range(NB):
                for cb in range(4):
                    pt = pixT_src if cb < 2 else pixT_ref
                    lhs = pt[:, jb, (cb % 2) * P : (cb % 2 + 1) * P]
                    nc.tensor.matmul(
                        psos[cb], lhs, f_sb[:, jb, :],
                        start=(jb == 0), stop=(jb == NB - 1),
                    )
            return psos

        def copy_out(s, psos):
            sl = ts(s, SLICE)
            for cb in range(4):
                nc.vector.tensor_copy(out=o_sb[:, cb, sl], in_=psos[cb])

        def finalize(lo, hi, dma_engines, skip_norm=False):
            """Normalize + blend + store for pixel columns [lo:hi).

            All elementwise work stays on DVE: GpSimd shares (and locks) the
            DVE SBUF port, so splitting across both engines makes each ~3x
            slower with no net gain.
            """
            r = slice(lo, hi)
            if not skip_norm:
                for cb in range(4):
                    nc.vector.tensor_mul(
                        o_sb[:, cb, r], o_sb[:, cb, r], invz_rep[:, r]
                    )
            for cb in (2, 3):
                ci = cb - 2
                nc.vector.tensor_sub(tmp[:, r], refb[:, ci, r], o_sb[:, cb, r])
                nc.vector.tensor_mul(tmp[:, r], tmp[:, r], mask_rep[:, r])
                nc.vector.tensor_add(o_sb[:, cb, r], o_sb[:, cb, r], tmp[:, r])
            # out rows: [flow(=cb2,3), src_att(=cb0,1)]
            for k, cb in enumerate([2, 3, 0, 1]):
                eng = dma_engines[k % len(dma_engines)]
                for s in range(lo // SLICE, (hi + SLICE - 1) // SLICE):
                    sl = ts(s, SLICE)
                    eng.dma_start(out=out_r[k, :, sl], in_=o_sb[:, cb, sl])

        with tc.tile_pool(name="obuf", bufs=1) as obuf:
            o_sb = obuf.tile([P, 4, HW], F32)
            tmp = obuf.tile([P, HW], F32)
            with tc.tile_pool(name="fbuf", bufs=1) as fbuf:
                for s in range(NS - 1):
                    f_sb = fbuf.tile([P, NB, SLICE], BF16, name="f_sb", tag="f")
                    scores_and_exp(s, f_sb)
                    psos = apply_mm(s, f_sb)
                    copy_out(s, psos)
                # slice 7: scores/exp first so Z completes, then overlap
                # finalize(0..6) with slice 7's apply matmuls
                s7 = NS - 1
                f_sb7 = fbuf.tile([P, NB, SLICE], BF16, name="f_sb", tag="f")
                scores_and_exp(s7, f_sb7)
                nc.vector.reduce_sum(out=z_all, in_=zpart, axis=AX_X)
                nc.vector.reciprocal(out=invz, in_=z_all)
                # broadcast invz[pixel] across the 128 channel partitions:
                # PE-transpose [128p, 32b] -> [32b, 128p], bounce through
                # DRAM contiguously, then partition-broadcast back (a direct
                # scatter of the untransposed layout costs ~16us)
                ps_t = ps_s.tile([NB, P], F32, name="ps_t", tag="pss")
                nc.tensor.transpose(ps_t, invz[:, :], ident)
                nc.vector.tensor_copy(out=invz_T, in_=ps_t)
                nc.sync.dma_start(
                    out=zrow.rearrange("(b q) -> b q", q=P), in_=invz_T
                )
                for s in range(NS):
                    sl = ts(s, SLICE)
                    nc.sync.dma_start(
                        out=invz_rep[:, sl], in_=zrow[sl].partition_broadcast(P)
                    )
                psos7 = apply_mm(s7, f_sb7)
                finalize(0, (NS - 1) * SLICE, [nc.sync, nc.scalar])
                # slice-7 copy-out doubles as its normalize (Z is ready now)
                sl7 = ts(s7, SLICE)
                for cb in range(4):
                    nc.vector.tensor_mul(
                        o_sb[:, cb, sl7], psos7[cb], invz_rep[:, sl7]
                    )
                finalize((NS - 1) * SLICE, HW, [nc.scalar, nc.sync], skip_norm=True)


def build():
    nc = bacc.Bacc(
        "TRN2",
        target_bir_lowering=False,
        debug=False,
        enable_asserts=False,
        num_devices=NCORES,
    )
    src = nc.dram_tensor("src", (C, HW), F32, kind="ExternalInput")
    ref = nc.dram_tensor("ref", (C, HW), F32, kind="ExternalInput")
    mask = nc.dram_tensor("mask", (HW,), F32, kind="ExternalInput")
    wT = nc.dram_tensor("wT", (C, CQ), BF16, kind="ExternalInput")
    out = nc.dram_tensor("out", (2 * C, HW), F32, kind="ExternalOutput")
    with tile.TileContext(nc) as tc:
        _build_body(tc, src, ref, mask, wT, out)
    nc.compile()
    return nc


_CACHE = {}


def _get_nc():
    if "nc" not in _CACHE:
        _CACHE["nc"] = build()
    return _CACHE["nc"]


def _in_maps(src_mask, src_feature, ref_feature, conv_w):
    import ml_dtypes

    n_batch = src_feature.shape[0]
    wT = np.ascontiguousarray(
        np.asarray(conv_w, dtype=np.float32).T.astype(ml_dtypes.bfloat16)
    )
    maps = []
    for n in range(n_batch):
        maps.append(
            {
                "src": np.ascontiguousarray(
                    np.asarray(src_feature[n], dtype=np.float32).reshape(C, HW)
                ),
                "ref": np.ascontiguousarray(
                    np.asarray(ref_feature[n], dtype=np.float32).reshape(C, HW)
                ),
                "mask": np.ascontiguousarray(
                    np.asarray(src_mask[n], dtype=np.float32).reshape(HW)
                ),
                "wT": wT,
            }
        )
    return maps


def _install_ntff_hook():
    """The agent image's antenv lacks axon_hooks; recreate it so
    run_bass_kernel_spmd(trace=True) can capture NTFF profiles."""
    import sys
    import types

    if "antenv.axon_hooks" in sys.modules:
        return
    import antenv
    from trn_agent_boot.trn_boot import _ntff_profile_via_ctypes

    hook = _ntff_profile_via_ctypes("/opt/axon/libaxon_pjrt.so")
    mod = types.ModuleType("antenv.axon_hooks")
    mod._hook = hook
    mod.set_axon_ntff_profile_hook = lambda h: setattr(mod, "_hook", h)
    mod.get_axon_ntff_profile_hook = lambda: mod._hook
    sys.modules["antenv.axon_hooks"] = mod
    antenv.axon_hooks = mod


def run(src_mask, src_feature, ref_feature, conv_w, trace=False):
    """Run on 8 NeuronCores. Returns (output [N,2C,H,W], BassKernelResults)."""
    n_batch, c, h, w = src_feature.shape
    if trace:
        _install_ntff_hook()
    nc = _get_nc()
    maps = _in_maps(src_mask, src_feature, ref_feature, conv_w)
    res = bass_utils.run_bass_kernel_spmd(
        nc, maps, core_ids=list(range(NCORES)), trace=trace
    )
    out = np.stack([r["out"] for r in res.results], axis=0)
    return out.reshape(n_batch, 2 * c, h, w).astype(np.float32), res


def kernel(src_mask, src_feature, ref_feature, conv_w):
    out, _ = run(src_mask, src_feature, ref_feature, conv_w)
    return out

